# revision 24
# baseline (speedup 1.0000x reference)
"""Trainium2 Bass kernel for MinibatchDiscrimination (symmetric scheme).

Reference computation:
    M = (x @ T.reshape(A, B*C)).reshape(N, B, C)        x:[2048,512] T:[512,64,16]
    O[i, b] = sum_j exp(-sum_c |M[j,b,c] - M[i,b,c]|)    O:[2048,64]

K(i,j) is symmetric, so each unordered pair is computed once and credited to
BOTH row-sums.  Core c owns rows R_c = [256c, 256c+256) and computes K against
a 1280-column local window (self block + next 3 blocks + the antipodal block
at half weight):

  - window cols [0, 1024)  = blocks c..c+3:  rowsum via exp accum_out; cols
    [256, 1024) also feed a column-sum (the partner row's credit).  Self-block
    cols [0, 256) are excluded from the column-sum because intra-block pairs
    already appear in both orientations inside the block.
  - window cols [1024, 1280) = antipodal block c+4: both cores of an antipodal
    pair compute the full block, so the psum D gets +ln2 added via a rank-1
    matmul (exp(-(D+ln2)) = 0.5 exp(-D)), credited to rowsum and column-sum.

Host combine: O[g] = rowsum[g] + sum over cores of colsum contributions
(local window col L+256 of core c maps to global row (256c+256+L) mod 2048).

Implementation notes:
  - Host feeds ONE bf16 input tensor per core: [xT rotated (1280 cols) | Tm
    (1024) | Sbig bits (120) | SelDR fp8 bits (128) | SelCS (64) | pad (2) |
    Ln2I (128)].  bf16 inputs make the M=x@T prologue matmuls 4x cheaper than
    fp32 and halve the input DMA.
  - MT = (x@T)^T as [bc=1024 -> 8 t-blocks x 128 partitions, j=1280] bf16.
  - Main loop over 128 i-pairs: absdiff slabs |MT - MT[:,i]| produced by ACT
    (Abs activation with per-partition bias, fp8 out for t-blocks 0..1, which
    lets the PE contract c with fp8 DoubleRow matmuls at 0.5 cyc/row; DR dst
    must start at partition 0, so pair-row ii=1 uses two normal fp8 matmuls)
    and DVE (tensor_scalar add-neg then bitwise_and sign-clear, bf16; one
    slab is j-split between the engines for balance).  PE contracts c(16)
    with 0/1 indicator matmuls into psum [2i x 64b, 1280] and adds ln2 to the
    antipodal region via a rank-1 matmul; ACT does one Exp(-D) -> SBUF bf16
    with accum_out writing the rowsum column directly; PE then reduces the
    exp tile over the two 64-partition groups into the persistent column-sum
    psum accumulators.
  - Sync discipline: every engine instruction carries at most ONE sem wait
    (walrus limit).  Cross-engine deps are subsumed through per-pair absorber
    instructions that advance each engine's clock (baseline-proven pattern).
"""

import numpy as np
import ml_dtypes

N, A_DIM, B, C = 2048, 512, 64, 16
BC = B * C
N_CORES = 8
SHARD = N // N_CORES          # 256
PAIRS = SHARD // 2            # 128
W1 = 1024                     # self + next 3 blocks
W2 = 256                      # antipodal block (half weight)
W = W1 + W2                   # 1280

XT_W = W
TM_OFF = XT_W                 # 1280
SB_OFF = TM_OFF + BC          # 2304
SELDR_OFF = SB_OFF + 120      # 2424
SELCS_OFF = SELDR_OFF + 128   # 2552
LN2I_OFF = SELCS_OFF + 64     # 2616
TOT_W = LN2I_OFF + 128        # 2744

# absdiff slab producers; t-block = 2s+kt.  s=0 tiles are fp8 (PE DoubleRow),
# s>=1 tiles bf16.  Slab (1,0,1) is j-split: DVE does [0, SPLIT), ACT the rest.
ACT_SLABS = ((0, 0, 0), (0, 0, 1), (0, 1, 0), (0, 1, 1), (1, 0, 0))
SPLIT_SLAB = (1, 0, 1)
SPLIT = 944
FP8_S = {0}

CHUNKS = ((0, 512), (512, 512), (1024, 256))

_CACHE = {}


def _build_nc(npairs=PAIRS):
    from contextlib import ExitStack
    import concourse.bass as bass
    import concourse.mybir as mybir
    from concourse.tile import TileContext, add_dep_helper

    f32 = mybir.dt.float32
    bf16 = mybir.dt.bfloat16
    fp8 = mybir.dt.float8e4
    u16 = mybir.dt.uint16
    Abs = mybir.ActivationFunctionType.Abs
    Exp = mybir.ActivationFunctionType.Exp
    Copy = mybir.ActivationFunctionType.Copy
    add_op = mybir.AluOpType.add
    band_op = mybir.AluOpType.bitwise_and
    DR = mybir.MatmulPerfMode.DoubleRow

    nc = bass.Bass("TRN2", target_bir_lowering=False, debug=False)
    XTm = nc.dram_tensor("XTm", [A_DIM, TOT_W], bf16, kind="ExternalInput").ap()
    # single output tensor: [:, :PAIRS] = rowsum columns (O_sb layout),
    # [0:64, PAIRS:] = column sums; one DMA -> one queue sem for the Drain
    OUT = nc.dram_tensor("OUT", [128, PAIRS + W - 256], f32,
                         kind="ExternalOutput").ap()

    with TileContext(nc) as tc, ExitStack() as ctx:
        singles = ctx.enter_context(tc.tile_pool(name="singles", bufs=1))

        XAll = singles.tile([128, 4, TOT_W], bf16, name="XAll", tag="XAll")
        nc.sync.dma_start(out=XAll, in_=XTm.rearrange("(k p) n -> p k n", p=128))

        Sbig = singles.tile([128, 120], bf16, name="Sbig", tag="Sbig")
        nc.scalar.copy(Sbig, XAll[:, 0, SB_OFF:SB_OFF + 120])
        SelDR = singles.tile([128, 2, 128], fp8, name="SelDR", tag="SelDR")
        nc.scalar.copy(SelDR.rearrange("p a b -> p (a b)"),
                       XAll[:, 0, SELDR_OFF:SELDR_OFF + 128].bitcast(fp8))
        SelCS = singles.tile([128, 64], bf16, name="SelCS", tag="SelCS")
        nc.scalar.copy(SelCS, XAll[:, 0, SELCS_OFF:SELCS_OFF + 64])
        Ln2I = singles.tile([128, 128], bf16, name="Ln2I", tag="Ln2I")
        nc.scalar.copy(Ln2I, XAll[:, 0, LN2I_OFF:LN2I_OFF + 128])
        ones = singles.tile([128, W2], bf16, name="ones", tag="ones")
        nc.scalar.activation(ones, XAll[:, 0, 0:W2], Copy, bias=1.0, scale=0.0)

        MT = singles.tile([128, 8, W], bf16, name="MT", tag="MT")
        negMTi = singles.tile([128, 8, SHARD], f32, name="negMTi", tag="negMTi")
        OutS = singles.tile([128, PAIRS + W - 256], f32, name="OutS", tag="OutS")
        O_sb = OutS[:, 0:PAIRS]
        csS = OutS[:, PAIRS:]
        mask16 = singles.tile([128, 1], u16, name="mask16", tag="mask16")
        nc.vector.memset(mask16, 0x7FFF)
        junkD = singles.tile([1, PAIRS], f32, name="junkD", tag="junkD")
        junkA = singles.tile([1, PAIRS], f32, name="junkA", tag="junkA")
        junkE = singles.tile([1, PAIRS], f32, name="junkE", tag="junkE")
        junkX = singles.tile([1, PAIRS], f32, name="junkX", tag="junkX")

        # cs banks first (banks 0-1), then the work pool (banks 2-7).  The
        # prologue shares the work pool's tiles so no cross-scope psum reuse
        # ever happens (scope-boundary syncs would add over-limit waits).
        with tc.tile_pool(name="cspsum", bufs=1, space="PSUM") as cspool, \
             tc.tile_pool(name="mpsum", bufs=2, space="PSUM") as mpsum, \
             tc.tile_pool(name="apool", bufs=2) as apool, \
             tc.tile_pool(name="epool", bufs=2) as epool:
            cs0 = cspool.tile([128, 512], f32, name="cs0", tag="cs0")
            cs1 = cspool.tile([128, 512], f32, name="cs1", tag="cs1")

            # ---- prologue: MT = (x @ T)^T in bf16 (psum f32), then negMTi.
            # psum->SBUF copies alternate ACT/DVE to halve the serial cost.
            last_pro_dve = None
            ci = 0
            for m in range(8):
                for (clo, cw) in CHUNKS:
                    ps = mpsum.tile([128, 1536], f32, name="ps", tag="ps")
                    for k in range(4):
                        nc.tensor.matmul(
                            ps[:, :cw],
                            XAll[:, k, TM_OFF + 128 * m:TM_OFF + 128 * (m + 1)],
                            XAll[:, k, clo:clo + cw],
                            start=(k == 0), stop=(k == 3),
                        )
                    if ci % 2 == 0:
                        nc.scalar.copy(MT[:, m, clo:clo + cw], ps[:, :cw])
                    else:
                        last_pro_dve = nc.vector.tensor_copy(
                            MT[:, m, clo:clo + cw], ps[:, :cw])
                    ci += 1
                last_pro_act = nc.scalar.mul(negMTi[:, m, :], MT[:, m, 0:SHARD], -1.0)

            prev_mm_last = {}      # q -> last matmul handle of pair q
            prev_dve_last = {}     # q -> last DVE absdiff handle of pair q
            prev_exp = {}          # q -> (exp handle, expS tile)
            for q in range(npairs):
                cols = (2 * q, 2 * q + 1)

                # --- A tiles for this pair
                A = {}
                for s in range(4):
                    dt_s = fp8 if s in FP8_S else bf16
                    for ii in range(2):
                        A[(s, ii)] = apool.tile([128, 2, W], dt_s,
                                                name=f"A{s}_{ii}", tag=f"A{s}_{ii}",
                                                bufs=2)

                # --- ACT absorbers: advance ACT's PE clock (A-tile WAR vs
                # pair q-2 matmul readers) and DVE clock (A-tile WAW vs pair
                # q-2 DVE writers; at q=0, the DVE prologue MT copies)
                act_markers = []
                if q >= 2:
                    absorber_pa = nc.scalar.copy(junkA[:, q:q + 1],
                                                 negMTi[0:1, 1, q:q + 1])
                    add_dep_helper(absorber_pa.ins, prev_mm_last[q - 2].ins,
                                   sync=True, reason="advance ACT PE-clock")
                    absorber_pd = nc.scalar.copy(junkE[:, q:q + 1],
                                                 negMTi[0:1, 0, q:q + 1])
                    add_dep_helper(absorber_pd.ins, prev_dve_last[q - 2].ins,
                                   sync=True, reason="advance ACT DVE-clock")
                    act_markers += [absorber_pa, absorber_pd]
                elif q == 0 and last_pro_dve is not None:
                    absorber_p0 = nc.scalar.copy(junkE[:, 0:1],
                                                 negMTi[0:1, 0, 0:1])
                    add_dep_helper(absorber_p0.ins, last_pro_dve.ins, sync=True,
                                   reason="ACT sees DVE prologue MT copies")
                    act_markers.append(absorber_p0)

                # --- ACT absdiff slabs
                for (s, ii, kt) in ACT_SLABS:
                    t = 2 * s + kt
                    act_i = nc.scalar.activation(A[(s, ii)][:, kt, :], MT[:, t, :],
                                                 Abs,
                                                 bias=negMTi[:, t, cols[ii]:cols[ii] + 1])
                    for mk in act_markers:
                        add_dep_helper(act_i.ins, mk.ins, sync=False,
                                       reason="order absdiff after ACT absorbers")
                # ACT part of the split slab
                s, ii, kt = SPLIT_SLAB
                t = 2 * s + kt
                act_i = nc.scalar.activation(
                    A[(s, ii)][:, kt, SPLIT:], MT[:, t, SPLIT:], Abs,
                    bias=negMTi[:, t, cols[ii]:cols[ii] + 1])
                for mk in act_markers:
                    add_dep_helper(act_i.ins, mk.ins, sync=False,
                                   reason="order absdiff after ACT absorbers")

                # --- DVE absorber: advance DVE's PE clock for A-tile WAR
                absorber_dv = None
                if q != 1:
                    absorber_dv = nc.vector.tensor_copy(junkD[:, q:q + 1],
                                                        negMTi[0:1, 0, q:q + 1])
                    dep = last_pro_act if q == 0 else prev_mm_last[q - 2]
                    add_dep_helper(absorber_dv.ins, dep.ins, sync=True,
                                   reason="advance DVE clock (prologue/PE WAR)")

                # --- DVE absdiff slabs: add-neg then joint sign-clear
                act_set = set(ACT_SLABS)
                dve_last = None
                for s in range(4):
                    for ii in range(2):
                        kts = [kt for kt in range(2) if (s, ii, kt) not in act_set]
                        if not kts:
                            continue
                        hi = {}
                        for kt in kts:
                            t = 2 * s + kt
                            hi[kt] = W if (s, ii, kt) != SPLIT_SLAB else SPLIT
                            ts = nc.vector.tensor_scalar(
                                A[(s, ii)][:, kt, 0:hi[kt]], MT[:, t, 0:hi[kt]],
                                negMTi[:, t, cols[ii]:cols[ii] + 1], None, op0=add_op)
                            if absorber_dv is not None:
                                add_dep_helper(ts.ins, absorber_dv.ins, sync=False,
                                               reason="order after DVE absorber")
                        if len(kts) == 2 and hi[kts[0]] == W and hi[kts[1]] == W:
                            view = A[(s, ii)].bitcast(u16)
                            dve_last = nc.vector.tensor_scalar(
                                view, view, mask16, None, op0=band_op)
                        else:
                            for kt in kts:
                                view = A[(s, ii)][:, kt, 0:hi[kt]].bitcast(u16)
                                dve_last = nc.vector.tensor_scalar(
                                    view, view, mask16, None, op0=band_op)

                # --- c-contraction matmuls into psum [2i x 64b, W]
                ps = mpsum.tile([128, 1536], f32, name="ps", tag="ps")
                if q >= 2:
                    # dummy matmul absorbs the psum-WAW PE-sem wait (the bank's
                    # last writer was a pair q-2 matmul; exp only reads psum)
                    dmy = nc.tensor.matmul(ps[0:32, 0:1], Sbig[:, 0:32],
                                           Sbig[:, 1:2], start=True, stop=True,
                                           skip_group_check=True)
                    add_dep_helper(dmy.ins, prev_mm_last[q - 2].ins, sync=True,
                                   reason="advance PE own-clock for psum WAW")
                first_group = True
                for ii in range(2):
                    chunk_order = CHUNKS if ii == 0 else (CHUNKS[0], CHUNKS[2], CHUNKS[1])
                    for (clo, cw) in chunk_order:
                        out = ps[64 * ii:64 * (ii + 1), clo:clo + cw]
                        if ii == 0:
                            # DoubleRow (2 k-tiles/instr) is only valid with
                            # dst partition base 0
                            nc.tensor.matmul(
                                out, SelDR[:, :, 64:128],
                                A[(0, ii)][:, :, clo:clo + cw],
                                start=True, stop=False, perf_mode=DR)
                        else:
                            nc.tensor.matmul(
                                out, SelDR[:, 0, 64:128],
                                A[(0, ii)][:, 0, clo:clo + cw],
                                start=True, stop=False)
                            nc.tensor.matmul(
                                out, SelDR[:, 1, 64:128],
                                A[(0, ii)][:, 1, clo:clo + cw],
                                start=False, stop=False)
                        for s in range(1, 4):
                            for kt in range(2):
                                t = 2 * s + kt
                                nc.tensor.matmul(
                                    out, Sbig[:, 56 - 8 * t:120 - 8 * t],
                                    A[(s, ii)][:, kt, clo:clo + cw],
                                    start=False, stop=(s == 3 and kt == 1))
                        if first_group and q >= 1:
                            # column-sum matmuls for pair q-1 (expS ready)
                            eprev = prev_exp[q - 1][1]
                            nc.tensor.matmul(cs0[0:64], SelCS, eprev[:, 256:768],
                                             start=(q == 1), stop=False,
                                             skip_group_check=True)
                            nc.tensor.matmul(cs1[0:64], SelCS, eprev[:, 768:1280],
                                             start=(q == 1), stop=False,
                                             skip_group_check=True)
                        first_group = False
                # rank-1 +ln2 on the antipodal region: exp then yields
                # 0.5*exp(-D) there (Ln2I has ln2 on partition 0 only)
                mm_last = nc.tensor.matmul(ps[:, W1:W], Ln2I, ones,
                                           start=False, stop=True,
                                           skip_group_check=True)

                # --- exp -> SBUF bf16; accum_out writes the rowsum column.
                # The absorber right before it carries the own-ACT sem wait of
                # the previous exp's accum aux (shared OutS tile WAW), pinned
                # late so it never stalls the pipeline.
                if q >= 1:
                    absorber_ex = nc.scalar.copy(junkX[:, q:q + 1],
                                                 negMTi[0:1, 2, q:q + 1])
                    add_dep_helper(absorber_ex.ins, prev_exp[q - 1][0].ins,
                                   sync=True, reason="advance ACT own accum clock")
                    add_dep_helper(absorber_ex.ins, act_i.ins, sync=False,
                                   reason="pin after this pair's absdiffs")
                expS = epool.tile([128, W], bf16, name="expS", tag="expS", bufs=2)
                exp_i = nc.scalar.activation(expS, ps[:, 0:W], Exp,
                                             scale=-1.0,
                                             accum_out=O_sb[:, q:q + 1])
                prev_exp[q] = (exp_i, expS)
                prev_mm_last[q] = mm_last
                prev_dve_last[q] = dve_last

            # column-sums of the last pair
            elast = prev_exp[npairs - 1][1]
            nc.tensor.matmul(cs0[0:64], SelCS, elast[:, 256:768],
                             start=(npairs == 1), stop=True, skip_group_check=True)
            nc.tensor.matmul(cs1[0:64], SelCS, elast[:, 768:1280],
                             start=(npairs == 1), stop=True,
                             skip_group_check=True)

            nc.scalar.copy(csS[0:64, 0:512], cs0[0:64])
            nc.scalar.copy(csS[0:64, 512:1024], cs1[0:64])
            dma_o = nc.sync.dma_start(out=OUT, in_=OutS)

    # Kernel-tail Drain aggregates one wait per active proc, exceeding the
    # CTRL struct's wait slots.  Every proc's completion is transitively
    # dominated by the output DMA; wait only on its queue sem.  The out-DMA
    # itself keeps only its ACT wait (the csS copies follow every exp and
    # the PE cs matmuls transitively).
    out_upd = {u.ant_name for u in dma_o.ins.sync_info.on_update}
    for f in nc.m.functions:
        for bb in f.blocks:
            for ins in bb.instructions:
                si = getattr(ins, 'sync_info', None)
                if si is None or len(si.on_wait) <= 1:
                    continue
                if ins.opcode == 'Drain':
                    kept = [w for w in si.on_wait if w.ant_name in out_upd]
                    assert kept, f"drain {ins.name} has no output-queue wait"
                    si.on_wait = kept
                elif ins.opcode == 'DMACopy':
                    kept = [w for w in si.on_wait if 'Activation' in w.ant_name]
                    assert kept, f"dma {ins.name} has no ACT wait"
                    si.on_wait = kept
    return nc


def _host_inputs(x, T):
    bf = ml_dtypes.bfloat16
    xT = np.ascontiguousarray(np.asarray(x, dtype=np.float32).T).astype(bf)   # [512, 2048]
    Tm = np.asarray(T, dtype=np.float32).reshape(A_DIM, BC).astype(bf)

    consts = np.zeros((A_DIM, TOT_W - SB_OFF), dtype=bf)
    # Sbig: [128, 120] with Sb[p, 56 + p//16] = 1 (windows for bf16 matmuls)
    for p in range(128):
        consts[p, 56 + p // 16] = 1
    # SelDR: [128, 2, 128] fp8, sel[p, kt, 64 + 8kt + p//16] = 1 (s=0 window)
    sel = np.zeros((128, 2, 128), dtype=ml_dtypes.float8_e4m3)
    for p in range(128):
        for kt in range(2):
            sel[p, kt, 64 + 8 * kt + p // 16] = 1
    selb = sel.reshape(128, 256).view(np.uint16).view(bf)                      # [128, 128]
    consts[:128, SELDR_OFF - SB_OFF:SELCS_OFF - SB_OFF] = selb
    # SelCS: [128, 64] with selcs[p, p % 64] = 1
    for p in range(128):
        consts[p, SELCS_OFF - SB_OFF + (p % 64)] = 1
    # Ln2I: [128, 128], partition 0 = ln2 (rank-1 bias matmul lhsT)
    consts[0, LN2I_OFF - SB_OFF:LN2I_OFF - SB_OFF + 128] = np.log(2.0)

    in_maps = []
    for c in range(N_CORES):
        xT_rot = np.roll(xT, -c * SHARD, axis=1)[:, :W]
        XTmc = np.ascontiguousarray(
            np.concatenate([xT_rot, Tm, consts], axis=1))
        in_maps.append({"XTm": XTmc})
    return in_maps


def run(x, T, npairs=PAIRS, trace=False):
    from concourse.bass_utils import run_bass_kernel_spmd

    nc = _CACHE.get(npairs)
    if nc is None:
        nc = _build_nc(npairs)
        _CACHE[npairs] = nc
    in_maps = _host_inputs(x, T)
    res = run_bass_kernel_spmd(nc, in_maps, list(range(N_CORES)), trace=trace)
    parts = []
    for c in range(N_CORES):
        out = np.asarray(res.results[c]["OUT"], dtype=np.float32)
        r = out[:, :PAIRS]                                   # [64ii+b, q]
        parts.append(r.reshape(2, B, PAIRS).transpose(2, 0, 1).reshape(SHARD, B))
    O = np.concatenate(parts, axis=0)
    for c in range(N_CORES):
        CSc = np.asarray(res.results[c]["OUT"], dtype=np.float32)[0:B, PAIRS:]
        idx = (np.arange(CSc.shape[1]) + SHARD * c + 256) % N
        O[idx] += CSc.T
    return O, res


def kernel(x, T):
    O, _ = run(x, T)
    return O
